# revision 29
# baseline (speedup 1.0000x reference)
"""Trainium2 Bass kernel for nn_AttentionalGNN (SuperGlue-style GNN).

Self-contained: takes FULL inputs, shards over 8 NeuronCores, returns FULL
outputs (d0, d1).

Sharding: core c -> (b = c>>2, s = (c>>1)&1, h = c&1): batch b, stream s
(desc0/desc1), n-half h. Each core owns x chunk [256, 512] and computes its
chunk of every layer; per layer the cores exchange x chunks (attention is
permutation-invariant along the key axis so chunk order is irrelevant).

Numerics: the network is chaotic - per-op fp32 rounding noise alone puts any
two independent fp32 implementations ~1.9e-2 apart after 18 layers, and the
gate sits at 2e-2 - so the arithmetic (op set, accumulation order, dtypes,
fp32r only for layers >= 16) matches the validated baseline bit-for-bit;
everything else is scheduling:
 - self layers: pair AllReduce (CCE adds own+partner inline), one read +
   subtract; emission is dependency-ordered (q, K half 0, V tiles 0-3 and
   hp0 scores over m-tiles 0-3 all run on the local chunk) so the exchange
   latency hides behind ~20us of PE work;
 - cross layers: ONE 4-rank AllGather instead of two pair-AllGathers
   (collectives have a ~20us latency floor and serialize on the single cc
   stream); the two other-stream chunks are picked out with exact 0/1
   per-core multipliers (csel input), which is bit-exact and avoids the
   (a+b)-a rounding of the pair trick;
 - attention: all of a head-pair's score matmuls stream on the PE while the
   softmax chain (Act evac -> DVE max-fold from PSUM -> Pool
   partition_all_reduce -> DVE sub -> Act exp into a dedicated f32r-capable
   p tile) trails behind; PV matmuls for heads 0-1 start while heads 2-3
   are still in softmax, and the Wm (msg) kt=0 accumulation is emitted
   between the PV pairs to keep the PE streaming;
 - softmax uses exact per-column max (scores reach ~3.9e3 by layer 17, so
   max-subtraction cannot be dropped), sums via a ones-column folded into
   the PV matmul, 2-ULP reciprocal;
 - projection bias-adds run on DVE (tensor_scalar_add); the W2 bias +
   residual is one fused scalar_tensor_tensor; h1 keeps the fused
   bias+ReLU on Act.
"""

import numpy as np

import concourse.bass as bass
import concourse.bass_isa as bass_isa
import concourse.mybir as mybir
import concourse.tile as tile
from concourse import bacc, bass_utils

F32 = mybir.dt.float32
F32R = mybir.dt.float32r
AF = mybir.ActivationFunctionType
ALU = mybir.AluOpType

L = 18
LAYER_TYPES = ["self", "cross"] * 9
HEADS = 4
DH = 64
D = 256
N = 1024
NLOC = 512
BN_EPS = 1e-5

# weight-column layout inside the per-layer [128, WCOLS] SBUF tile
OFF_QT = 0      # [2, 256]
OFF_KT = 512    # [2, 256]
OFF_VT = 1024   # [2, 256]
OFF_MT = 1536   # [2, 256]
OFF_W1 = 2048   # [4, 512]
OFF_W2 = 4096   # [4, 256]
WCOLS = 5120

# bias columns in [128, 12]: bq(2) bk(2) bm(2) b1(4) b2(2)
BQ, BK, BM, B1, B2 = 0, 2, 4, 6, 10

R32_START = 16  # layers >= this run projections/MLP matmuls in float32r

GROUPS_P = [[0, 1], [2, 3], [4, 5], [6, 7]]        # same (b,s), other h
GROUPS_A = [[0, 2], [1, 3], [4, 6], [5, 7]]        # other s, same h
GROUPS_B = [[0, 3], [1, 2], [4, 7], [5, 6]]        # other s, other h
GROUPS_4 = [[0, 1, 2, 3], [4, 5, 6, 7]]            # whole batch group

_CACHE = {}


def _head_perm():
    # torch view(b, dim//h, h, n): channel c=(dh_idx*HEADS + head)
    # -> grouped g=(head*DH + dh_idx)
    perm = np.zeros(D, dtype=np.int64)
    for c in range(D):
        dh_idx, h = divmod(c, HEADS)
        perm[h * DH + dh_idx] = c
    return perm


def _prep_params(inputs):
    """Fold scale/BN/bv, permute heads; pack wts [L,128,WCOLS] bias [L,128,12]."""
    perm = _head_perm()
    wts = np.zeros((L, 128, WCOLS), np.float32)
    bias = np.zeros((L, 128, 12), np.float32)
    f32 = lambda a: np.asarray(a, np.float32)
    for i in range(L):
        Wq = f32(inputs["Wq"][i])[perm] / 8.0
        bq = f32(inputs["bq"][i])[perm] / 8.0
        Wk = f32(inputs["Wk"][i])[perm]
        bk = f32(inputs["bk"][i])[perm]
        Wv = f32(inputs["Wv"][i])[perm]
        bv = f32(inputs["bv"][i])[perm]
        Wm = f32(inputs["Wm"][i])[:, perm]
        bm = Wm @ bv + f32(inputs["bm"][i])
        scale = f32(inputs["gamma"][i]) / np.sqrt(f32(inputs["rv"][i]) + BN_EPS)
        W1 = f32(inputs["W1"][i]) * scale[:, None]
        b1 = (f32(inputs["b1"][i]) - f32(inputs["rm"][i])) * scale \
            + f32(inputs["beta"][i])
        W2 = f32(inputs["W2"][i])
        b2 = f32(inputs["b2"][i])

        def putT(W, off, osz):
            WT = W.T  # [in, out]
            kt_n = WT.shape[0] // 128
            for kt in range(kt_n):
                wts[i, :, off + kt * osz: off + (kt + 1) * osz] = \
                    WT[kt * 128:(kt + 1) * 128]

        putT(Wq, OFF_QT, 256)
        putT(Wk, OFF_KT, 256)
        putT(Wv, OFF_VT, 256)
        putT(Wm, OFF_MT, 256)
        putT(W1, OFF_W1, 512)
        putT(W2, OFF_W2, 256)
        for ot in range(2):
            bias[i, :, BQ + ot] = bq[ot * 128:(ot + 1) * 128]
            bias[i, :, BK + ot] = bk[ot * 128:(ot + 1) * 128]
            bias[i, :, BM + ot] = bm[ot * 128:(ot + 1) * 128]
            bias[i, :, B2 + ot] = b2[ot * 128:(ot + 1) * 128]
        for ot in range(4):
            bias[i, :, B1 + ot] = b1[ot * 128:(ot + 1) * 128]
    # pre-rounded (tf32-like RNE, drop 12 mantissa bits) late-layer weights
    u = wts[R32_START:].view(np.uint32)
    half = np.uint32(1 << 11)
    mask = np.uint32(0xFFFFF000)
    wtsr = ((u + half) & mask).view(np.float32).copy()
    return wts, bias, wtsr


def _build(n_layers=L, nocc=False):
    nc = bacc.Bacc("TRN2", target_bir_lowering=False, debug=False,
                   num_devices=8)
    x0 = nc.dram_tensor("x0", [2, 128, NLOC], F32, kind="ExternalInput").ap()
    wts = nc.dram_tensor("wts", [L, 128, WCOLS], F32,
                         kind="ExternalInput").ap()
    wtsr = nc.dram_tensor("wtsr", [L - R32_START, 128, WCOLS],
                          F32R, kind="ExternalInput").ap()
    bias = nc.dram_tensor("bias", [L, 128, 12], F32,
                          kind="ExternalInput").ap()
    csel = nc.dram_tensor("csel", [128, 2], F32, kind="ExternalInput").ap()
    y = nc.dram_tensor("y", [2, 128, NLOC], F32, kind="ExternalOutput").ap()

    with tile.TileContext(nc) as tc:
        with tc.tile_pool(name="wp", bufs=2) as wp, \
             tc.tile_pool(name="bp", bufs=2) as bp, \
             tc.tile_pool(name="xp", bufs=1) as xp, \
             tc.tile_pool(name="srcp", bufs=1) as srcp, \
             tc.tile_pool(name="kp", bufs=1) as kp, \
             tc.tile_pool(name="qp", bufs=1) as qp, \
             tc.tile_pool(name="vtp", bufs=1) as vtp, \
             tc.tile_pool(name="sp", bufs=3) as sp, \
             tc.tile_pool(name="pp", bufs=2) as pp, \
             tc.tile_pool(name="mp", bufs=1) as mp, \
             tc.tile_pool(name="small", bufs=4) as small, \
             tc.tile_pool(name="ps_proj", bufs=2, space="PSUM") as ps_proj, \
             tc.tile_pool(name="ps_sc", bufs=4, space="PSUM") as ps_sc, \
             tc.tile_pool(name="ps_pv", bufs=2, space="PSUM") as ps_pv, \
             tc.tile_pool(name="dram", bufs=1, space="DRAM") as dram:

            # tiny warmup collective: pays the cc cold-start cost while
            # the initial x/weight loads and local compute run
            warm_in = dram.tile([128, 2], F32, tag="warm_in")
            warm_out = dram.tile([2, 128, 2], F32, tag="warm_out")
            nc.gpsimd.collective_compute(
                "AllGather", ALU.bypass, replica_groups=GROUPS_P,
                ins=[warm_in.opt()], outs=[warm_out.opt()],
            )
            x_sb = xp.tile([128, 2, NLOC], F32)
            nc.sync.dma_start(x_sb[:], x0.rearrange("c p n -> p c n"))
            ones_t = xp.tile([128, 8, 4], F32)
            nc.vector.memset(ones_t[:], 1.0)
            cs_sb = xp.tile([128, 2], F32)  # col 0: cs, col 1: 1-cs
            nc.sync.dma_start(cs_sb[:], csel)


            def emit_ag(src_slice, groups, cc_in, first_layer=False):
                """AllGather over `groups`; dst = out[0]+out[1] (sub of own
                chunk is the caller's job). Returns nothing."""
                if first_layer:
                    cc_out = dram.tile([2, 2, 128, NLOC], F32, tag="cc_out0",
                                       bufs=1)
                else:
                    cc_out = dram.tile([2, 128, 2, NLOC], F32, tag="cc_out",
                                       bufs=4)
                nc.gpsimd.collective_compute(
                    "AllGather", ALU.bypass, replica_groups=groups,
                    ins=[cc_in], outs=[cc_out.opt()],
                )
                if first_layer:
                    nc.sync.dma_start(
                        src_slice,
                        cc_out[0].rearrange("c p n -> p c n"))
                    nc.gpsimd.dma_start(
                        src_slice,
                        cc_out[1].rearrange("c p n -> p c n"),
                        accum_op=ALU.add)
                else:
                    nc.sync.dma_start(src_slice, cc_out[0])
                    nc.gpsimd.dma_start(src_slice, cc_out[1],
                                        accum_op=ALU.add)

            for li in range(n_layers):
                ltype = LAYER_TYPES[li]
                lp = li >= R32_START
                rd = (lambda ap: ap.bitcast(F32R)) if lp else (lambda ap: ap)
                # ---- exchange current x chunks (emitted BEFORE the
                # weight prefetch: the cc_in staging and the 2.6MB weight
                # load share the sync DMA queue, and staging must reach the
                # collective trigger first -- the weight load then drains
                # inside the AllGather's dead latency window) ----
                # src0/src1 are the two key-halves fed to K/V.  For self
                # layers src0 is the local x (read in place), src1 comes from
                # the h-partner.  For cross layers both come from the two
                # other-stream cores; one staged cc_in feeds both AllGathers
                # so their transfers pipeline in the cc stream.
                # f32r copy of x for q-proj / W1 rhs (and self-layer src0)
                # on late layers: fp32r matmuls need f32r-rounded inputs
                if lp:
                    x_r = small.tile([128, 2, NLOC], F32R, tag="xr", bufs=1)
                    nc.vector.tensor_copy(x_r[:], x_sb[:])
                    x_use = x_r
                else:
                    x_use = x_sb
                src1 = srcp.tile([128, 2, NLOC], F32, tag="src1")
                if nocc:
                    src0 = srcp.tile([128, 2, NLOC], F32, tag="src0")
                    nc.vector.tensor_copy(src0[:], x_sb[:])
                    nc.vector.tensor_copy(src1[:], x_sb[:])
                elif ltype == "self":
                    src0 = x_r if lp else x_sb
                    if li == 0:
                        cc_in0 = dram.tile([2, 128, NLOC], F32, tag="cc_in0")
                        nc.sync.dma_start(cc_in0[:], x0)
                        emit_ag(src1[:], GROUPS_P, cc_in0.opt(),
                                first_layer=True)
                    else:
                        cc_in = dram.tile([128, 2, NLOC], F32, tag="cc_in",
                                          bufs=2)
                        nc.sync.dma_start(cc_in[:], x_sb[:])
                        emit_ag(src1[:], GROUPS_P, cc_in.opt())
                    nc.vector.tensor_tensor(rd(src1)[:], src1[:], x_sb[:],
                                            ALU.subtract)
                elif lp:
                    # fp32r layer: keep the pair-AG path (its DVE subtract
                    # rounds to f32r; the select path's STT cannot)
                    src0 = srcp.tile([128, 2, NLOC], F32, tag="src0")
                    cc_in = dram.tile([128, 2, NLOC], F32, tag="cc_in",
                                      bufs=2)
                    nc.sync.dma_start(cc_in[:], x_sb[:])
                    emit_ag(src0[:], GROUPS_A, cc_in.opt())
                    emit_ag(src1[:], GROUPS_B, cc_in.opt())
                    nc.vector.tensor_tensor(rd(src0)[:], src0[:], x_sb[:],
                                            ALU.subtract)
                    nc.vector.tensor_tensor(rd(src1)[:], src1[:], x_sb[:],
                                            ALU.subtract)
                else:
                    # one 4-rank AllGather; pick the two other-stream chunks
                    # with exact 0/1 per-core multipliers (bit-exact select)
                    src0 = srcp.tile([128, 2, NLOC], F32, tag="src0")
                    cc_in = dram.tile([128, 2, NLOC], F32, tag="cc_in",
                                      bufs=2)
                    nc.sync.dma_start(cc_in[:], x_sb[:])
                    cc4 = dram.tile([4, 128, 2, NLOC], F32, tag="cc4",
                                    bufs=2)
                    nc.gpsimd.collective_compute(
                        "AllGather", ALU.bypass, replica_groups=GROUPS_4,
                        ins=[cc_in.opt()], outs=[cc4.opt()],
                    )
                    # reads fan out over four DMA queues; src0's select
                    # runs per-kt-half so the K projection starts as soon as
                    # the first 512 columns are ready
                    t2 = srcp.tile([128, 2, NLOC], F32, tag="t2")
                    t3 = srcp.tile([128, 2, NLOC], F32, tag="t3")
                    nc.sync.dma_start(src0[:], cc4[0])
                    nc.scalar.dma_start(src1[:], cc4[1])
                    nc.gpsimd.dma_start(t2[:], cc4[2])
                    nc.sync.dma_start(t3[:], cc4[3])
                    for kt in range(2):
                        nc.vector.tensor_scalar_mul(src0[:, kt, :],
                                                    src0[:, kt, :],
                                                    cs_sb[:, 1:2])
                        nc.vector.scalar_tensor_tensor(
                            src0[:, kt, :], t2[:, kt, :], cs_sb[:, 0:1],
                            src0[:, kt, :], ALU.mult, ALU.add)
                    nc.vector.tensor_scalar_mul(src1[:], src1[:],
                                                cs_sb[:, 1:2])
                    nc.vector.scalar_tensor_tensor(
                        src1[:], t3[:], cs_sb[:, 0:1], src1[:],
                        ALU.mult, ALU.add)
                srcs = [src0 if (lp and ltype == "self" and not nocc)
                        else rd(src0), rd(src1)]

                wt = wp.tile([128, WCOLS], F32R if lp else F32, tag="wt")
                bt = bp.tile([128, 12], F32, tag="bt")
                nc.sync.dma_start(wt[:],
                                  wtsr[li - R32_START] if lp else wts[li])
                nc.sync.dma_start(bt[:], bias[li])

                # ---- q projection (local x only; overlaps the exchange) ----
                q_sb = qp.tile([128, 2, NLOC], F32, tag="q")
                for ot in range(2):
                    ps = ps_proj.tile([128, NLOC], F32, tag="proj")
                    for kt in range(2):
                        nc.tensor.matmul(
                            ps[:],
                            wt[:, OFF_QT + kt * 256 + ot * 128:
                                   OFF_QT + kt * 256 + (ot + 1) * 128],
                            x_use[:, kt, :],
                            start=(kt == 0), stop=(kt == 1))
                    nc.vector.tensor_scalar_add(q_sb[:, ot, :], ps[:],
                                                bt[:, BQ + ot:BQ + ot + 1])

                # ---- K/V/scores, ordered by src dependency ----
                # All src0-dependent work (K m-half 0, V m-tiles 0-3, hp0
                # scores over m-tiles 0-3) is emitted before anything that
                # needs src1, so on self layers the AllGather for src1 hides
                # behind ~20us of local compute.
                k_sb = kp.tile([128, 2, N], F32, tag="k")
                vt = vtp.tile([128, 8, 260], F32R if lp else F32, tag="vt")
                nc.vector.tensor_copy(
                    vt.rearrange("p m (h c) -> p m h c", c=65)[:, :, :, 64],
                    ones_t[:])

                def emit_k(mc):
                    for ot in range(2):
                        ps = ps_proj.tile([128, NLOC], F32, tag="proj",
                                          name=f"psk_{li}_{mc}_{ot}")
                        for kt in range(2):
                            nc.tensor.matmul(
                                ps[:],
                                wt[:, OFF_KT + kt * 256 + ot * 128:
                                       OFF_KT + kt * 256 + (ot + 1) * 128],
                                srcs[mc][:, kt, :],
                                start=(kt == 0), stop=(kt == 1))
                        nc.vector.tensor_scalar_add(
                            k_sb[:, ot, mc * NLOC:(mc + 1) * NLOC], ps[:],
                            bt[:, BK + ot:BK + ot + 1])

                def emit_v(mts):
                    for mt in mts:
                        ps = ps_proj.tile([128, 256], F32, tag="proj",
                                          name=f"psv_{li}_{mt}")
                        for kt in range(2):
                            nc.tensor.matmul(
                                ps[:],
                                srcs[mt // 4][:, kt,
                                              (mt % 4) * 128:
                                              (mt % 4 + 1) * 128],
                                wt[:, OFF_VT + kt * 256:
                                       OFF_VT + (kt + 1) * 256],
                                start=(kt == 0), stop=(kt == 1))
                        nc.scalar.copy(
                            vt.rearrange("p m (h c) -> p m h c",
                                         c=65)[:, mt, :, 0:64],
                            ps.rearrange("p (h c) -> p h c", c=64))

                s_t = [None] * 4
                p_t = [None] * 4
                acc_t = [None] * 4

                def emit_scores(hp, mts):
                    for hh in range(2):
                        h = hp * 2 + hh
                        if s_t[h] is None:
                            s_t[h] = sp.tile([128, 8, NLOC], F32, tag="s",
                                             name=f"s_{li}_{h}")
                            acc_t[h] = small.tile(
                                [128, NLOC], F32, tag="acc", bufs=2,
                                name=f"acc_{li}_{h}")
                    for mt in mts:
                        for hh in range(2):
                            h = hp * 2 + hh
                            base = 64 * hh
                            ps = ps_sc.tile([128, NLOC], F32, tag="sc",
                                            name=f"pssc_{li}_{hp}_{mt}_{hh}")
                            nc.tensor.matmul(
                                ps[:],
                                k_sb[base:base + 64, hp,
                                     mt * 128:(mt + 1) * 128],
                                q_sb[base:base + 64, hp, :],
                                start=True, stop=True,
                                tile_position=(base, 0))
                            nc.scalar.copy(s_t[h][:, mt, :], ps[:])
                            if mt == 0:
                                nc.vector.tensor_copy(acc_t[h][:], ps[:])
                            else:
                                nc.vector.tensor_tensor(
                                    acc_t[h][:], ps[:], acc_t[h][:],
                                    ALU.max)

                def emit_softmax(hp):
                    for hh in range(2):
                        h = hp * 2 + hh
                        gmax = small.tile([128, NLOC], F32, tag="gmax",
                                          bufs=2, name=f"gmax_{li}_{h}")
                        nc.gpsimd.partition_all_reduce(
                            gmax[:], acc_t[h][:], channels=128,
                            reduce_op=bass_isa.ReduceOp.max)
                        p_t[h] = pp.tile([128, 8, NLOC],
                                         F32R if lp else F32, tag="p",
                                         name=f"p_{li}_{h}")
                        for mt in range(8):
                            nc.vector.tensor_tensor(s_t[h][:, mt, :],
                                                    s_t[h][:, mt, :],
                                                    gmax[:], ALU.subtract)
                            nc.scalar.activation(p_t[h][:, mt, :],
                                                 s_t[h][:, mt, :], AF.Exp)

                emit_k(0)
                emit_v([0, 1, 2, 3])
                emit_scores(0, [0, 1, 2, 3])
                emit_k(1)
                emit_v([4, 5, 6, 7])
                emit_scores(0, [4, 5, 6, 7])
                emit_softmax(0)
                emit_scores(1, list(range(8)))
                emit_softmax(1)

                # Phase 3: PV per head + normalize; the Wm (msg) matmul's
                # kt=0 half is emitted as soon as heads 0-1 are normalized so
                # the PE keeps streaming while heads 2-3 finish softmax
                out_sb = small.tile([128, 2, NLOC], F32R if lp else F32,
                                    tag="out", bufs=1)

                def emit_pv(h):
                    hp, hh = h // 2, h % 2
                    base = 64 * hh
                    po = ps_pv.tile([65, NLOC], F32, tag="pv",
                                    name=f"po_{li}_{h}")
                    for mt in range(8):
                        nc.tensor.matmul(
                            po[:],
                            vt[:, mt, 65 * h:65 * h + 65],
                            p_t[h][:, mt, :],
                            start=(mt == 0), stop=(mt == 7))
                    sums_sb = small.tile([1, NLOC], F32, tag="sums", bufs=1,
                                         name=f"sums_{li}_{h}")
                    nc.scalar.copy(sums_sb[:], po[64:65, :])
                    rb = small.tile([1, NLOC], F32, tag="rb", bufs=1,
                                    name=f"rb_{li}_{h}")
                    scr = small.tile([1, NLOC], F32, tag="scr", bufs=1,
                                     name=f"scr_{li}_{h}")
                    nc.vector.reciprocal_approx_accurate(rb[:], sums_sb[:],
                                                         scr[:])
                    rbc = small.tile([64, NLOC], F32, tag="rbc", bufs=1,
                                     name=f"rbc_{li}_{h}")
                    nc.gpsimd.partition_broadcast(rbc[:], rb[0:1, :])
                    nc.vector.tensor_mul(out_sb[base:base + 64, hp, :],
                                         po[0:64, :], rbc[:])

                msg = small.tile([128, 2, NLOC], F32R if lp else F32,
                                 tag="msg", bufs=1)
                msg_ps = []
                for h in (0, 1):
                    emit_pv(h)
                for ot in range(2):
                    ps = ps_proj.tile([128, NLOC], F32, tag="proj",
                                      name=f"psm_{li}_{ot}")
                    msg_ps.append(ps)
                    nc.tensor.matmul(
                        ps[:],
                        wt[:, OFF_MT + ot * 128:OFF_MT + (ot + 1) * 128],
                        out_sb[:, 0, :], start=True, stop=False)
                for h in (2, 3):
                    emit_pv(h)
                for ot in range(2):
                    ps = msg_ps[ot]
                    nc.tensor.matmul(
                        ps[:],
                        wt[:, OFF_MT + 256 + ot * 128:
                               OFF_MT + 256 + (ot + 1) * 128],
                        out_sb[:, 1, :], start=False, stop=True)
                    nc.vector.tensor_scalar_add(msg[:, ot, :], ps[:],
                                                bt[:, BM + ot:BM + ot + 1])

                # ---- h1 = relu(W1' @ [x; msg] + b1') ----
                h1 = mp.tile([128, 4, NLOC], F32R if lp else F32, tag="h1")
                cat = [x_use[:, 0, :], x_use[:, 1, :], msg[:, 0, :],
                       msg[:, 1, :]]
                for ot in range(4):
                    ps = ps_proj.tile([128, NLOC], F32, tag="proj")
                    for kt in range(4):
                        nc.tensor.matmul(
                            ps[:],
                            wt[:, OFF_W1 + kt * 512 + ot * 128:
                                   OFF_W1 + kt * 512 + (ot + 1) * 128],
                            cat[kt],
                            start=(kt == 0), stop=(kt == 3))
                    nc.scalar.activation(h1[:, ot, :], ps[:], AF.Relu,
                                         bias=bt[:, B1 + ot:B1 + ot + 1])

                # ---- x += W2 @ h1 + b2  (fused bias+residual on DVE) ----
                for ot in range(2):
                    ps = ps_proj.tile([128, NLOC], F32, tag="proj")
                    for kt in range(4):
                        nc.tensor.matmul(
                            ps[:],
                            wt[:, OFF_W2 + kt * 256 + ot * 128:
                                   OFF_W2 + kt * 256 + (ot + 1) * 128],
                            h1[:, kt, :],
                            start=(kt == 0), stop=(kt == 3))
                    nc.vector.scalar_tensor_tensor(
                        x_sb[:, ot, :], ps[:], bt[:, B2 + ot:B2 + ot + 1],
                        x_sb[:, ot, :], ALU.add, ALU.add)

            nc.sync.dma_start(y.rearrange("c p n -> p c n"), x_sb[:])

    nc.compile()
    return nc


def get_nc(n_layers=L, nocc=False):
    key = (n_layers, nocc)
    if key not in _CACHE:
        _CACHE[key] = _build(n_layers, nocc)
    return _CACHE[key]


def kernel(**inputs):
    nc = get_nc()
    wts, bias, wtsr = _prep_params(inputs)
    d0 = np.ascontiguousarray(np.asarray(inputs["desc0"], np.float32))
    d1 = np.ascontiguousarray(np.asarray(inputs["desc1"], np.float32))
    descs = [d0, d1]
    in_maps = []
    for c in range(8):
        b, s, h = c >> 2, (c >> 1) & 1, c & 1
        chunk = descs[s][b][:, h * NLOC:(h + 1) * NLOC]  # [256, 512]
        cs = 1.0 if s == 0 else 0.0
        csel = np.zeros((128, 2), np.float32)
        csel[:, 0] = cs
        csel[:, 1] = 1.0 - cs
        in_maps.append({
            "x0": np.ascontiguousarray(chunk.reshape(2, 128, NLOC)),
            "wts": wts,
            "bias": bias,
            "wtsr": wtsr,
            "csel": csel,
        })
    res = bass_utils.run_bass_kernel_spmd(nc, in_maps,
                                          core_ids=list(range(8)))
    o0 = np.zeros((2, D, N), np.float32)
    o1 = np.zeros((2, D, N), np.float32)
    outs = [o0, o1]
    for c in range(8):
        b, s, h = c >> 2, (c >> 1) & 1, c & 1
        yc = res.results[c]["y"].reshape(D, NLOC)
        outs[s][b][:, h * NLOC:(h + 1) * NLOC] = yc
    return o0, o1
